# revision 61
# baseline (speedup 1.0000x reference)
"""Distributed Trainium2 kernel for nn_ARLoss_88390426951926.

Computes mean(loss) where, per element (EPS = 1e-6):
    c = round(t); d = x - c; pos = d >= 0
    z = pos ? ceil(x) : floor(x)
    loss = max(0, |d| - |x - z| + pos*EPS)

Algebraic reduction (validated to ~5e-6 rel err on the real data):
    With F = floor(x), S = 2x - c - F, S5 = S - 0.5:
        loss = relu(S - 1 + eps) + relu(-S)            (ties measure-zero)
             = max(S5, 0.5) + max(-S5, 0.5) - 1
so sum(loss) = sum max(S5,.5) - sum min(S5,-.5) - N.

Rounding uses the float32 magic-number trick (M = 1.5*2^23):
    c = (t + M) - M            (round-to-nearest-even)
    F = ((x - 0.5) + M) - M    (floor, up to RNE ties at exact integers)

Engine split per 128xFD tile (order matters: the long chain is
x -> y -> F -> e -> S5, so x's DMA is issued first and y/F run first):
    DVE TS2: y   = (x - 0.5) + M       f32->f32   (2x_2p, 0.5 cyc/elem)
    ACT A1 : F   = Copy(y - M)         f32->bf16
    DVE TS1: c   = (t + M) - M         f32->bf16  (2x_2p)
    ACT A2 : x2h = Copy(2x - 0.5)      f32->bf16
    DVE TT1: e   = c + F               bf16       (2x_1p)
    DVE TT2: S5  = x2h - e             bf16       (2x_1p)
    DVE TSp: qp  = max(S5, 0.5)        bf16       (4x_2p)
    PE     : psum[1,512] += ones.T @ qp            (idle engine)
    min-branch, alternating per segment to balance DVE vs ACT:
      DVE TSm: qm = min(S5, -0.5); psum += (-ones).T @ qm
      ACT AR : acc2[:,col] = sum relu(-S5 - 0.5)   (accum_out, full rate)
First and last tiles are split into 4 quarter tiles so the pipeline
ramps in and drains out quickly. DMA granularity stays at 2 MiB per
transfer, but compute runs in [128, 2048] half-tiles, halving the mid
pool so the x-input pool affords 5 buffers (~10 MiB of DMA lookahead);
this keeps the input stream gapless (measured: continuous 8.8->93.5us).
Per core measured: DVE ~93us busy, ACT ~83us, PE ~43us, DMA ~94us (HBM
roofline); exec ~118us = DMA + ~21us fixed Tile prologue/epilogue + ~3us
residual. Device intermittently clock-throttles ~20% (then ~140us).
Notes from hardware probing (this toolchain):
  - tensor_scalar with accum_out (cache-reduce form) runs at ~1.2
    cyc/elem regardless of dtype - do NOT use it on the hot path.
  - abs_max is not a valid ALU op on HW TensorScalar; max/min are.
  - ACT activation accum_out runs at full ACT pass rate.
"""

import sys
import types

import numpy as np

import concourse.bass as bass
import concourse.bacc as bacc
import concourse.mybir as mybir
from concourse.tile import TileContext
from concourse.bass_utils import run_bass_kernel_spmd


def _ensure_axon_hooks():
    """Some agent images lack ``antenv.axon_hooks``; if BASS_TRACE is set
    in the environment, run_bass_kernel_spmd imports it and would crash.
    Provide a no-op hook registry so tracing degrades gracefully."""
    try:
        import antenv  # noqa: F401
    except ImportError:
        return
    try:
        import antenv.axon_hooks  # noqa: F401
        return
    except ImportError:
        pass
    mod = types.ModuleType("antenv.axon_hooks")
    _state = {"hook": None}
    mod.set_axon_ntff_profile_hook = lambda h: _state.__setitem__("hook", h)
    mod.get_axon_ntff_profile_hook = lambda: _state["hook"]
    sys.modules["antenv.axon_hooks"] = mod
    import antenv as _a

    _a.axon_hooks = mod


_ensure_axon_hooks()

B, D = 8192, 4096
N_CORES = 8
ROWS = B // N_CORES          # 1024 rows per core
P = 128                      # SBUF partitions
FD = 4096                    # free dim per full tile
NTILES = (ROWS * D) // (P * FD)   # 8 full tiles per core
MAGIC = 12582912.0           # 1.5 * 2**23
MM_N = 512                   # matmul free-dim chunk (one PSUM bank)

F32 = mybir.dt.float32
BF16 = mybir.dt.bfloat16

HFD = FD // 2                # compute half-tile width (smaller mid pool)

# DMA segments: (tile_idx, fd_off, dma_fd). First/last tiles in quarters
# for fast ramp/drain; middle tiles as full 2 MiB transfers.
DMA_SEGS = [(0, s * (FD // 4), FD // 4) for s in range(4)]
DMA_SEGS += [(i, 0, FD) for i in range(1, NTILES - 1)]
DMA_SEGS += [(NTILES - 1, s * (FD // 4), FD // 4) for s in range(4)]

# Compute segments: (dma_idx, local_off, fd, use_act_min). Full tiles are
# computed in two half-tiles so mid intermediates are [P, HFD].
COMP_SEGS = []
for _di, (_ti, _off, _dfd) in enumerate(DMA_SEGS):
    _ua = _ti in (2, 4, 6) if _dfd == FD else (_ti == NTILES - 1)
    for _h in range(max(1, _dfd // HFD)):
        COMP_SEGS.append((_di, _h * HFD, min(HFD, _dfd), _ua))
N_COLS = len(COMP_SEGS)
# ACT relu-accum cells compute max(-S5,.5) - 0.5 per element.
ACT_MIN_ELEMS = sum(P * fd for (_, _, fd, ua) in COMP_SEGS if ua)

# Exposed for test.py: the BassKernelResults of the last run.
LAST_RESULTS = None
_CACHE = {}


def build_nc():
    nc = bacc.Bacc(dynamic_dma_scratch_size=512)
    x_d = nc.dram_tensor("input", [ROWS, D], F32, kind="ExternalInput")
    t_d = nc.dram_tensor("target", [ROWS, D], F32, kind="ExternalInput")
    qsum_d = nc.dram_tensor("qsum", [1, MM_N], F32, kind="ExternalOutput")
    acc2_d = nc.dram_tensor("acc2", [P, N_COLS], F32, kind="ExternalOutput")

    x_t = x_d[:, :].rearrange("(n p) m -> n p m", p=P)
    t_t = t_d[:, :].rearrange("(n p) m -> n p m", p=P)

    add = mybir.AluOpType.add
    sub = mybir.AluOpType.subtract
    amax = mybir.AluOpType.max
    amin = mybir.AluOpType.min
    Copy = mybir.ActivationFunctionType.Copy
    Relu = mybir.ActivationFunctionType.Relu

    n_matmuls = sum(
        (fd // MM_N) * (1 if ua else 2) for (_, _, fd, ua) in COMP_SEGS
    )

    with TileContext(nc) as tc:
        with (
            tc.tile_pool(name="iox", bufs=5) as iox_pool,
            tc.tile_pool(name="iot", bufs=3) as iot_pool,
            tc.tile_pool(name="mid", bufs=2) as mid_pool,
            tc.tile_pool(name="fix", bufs=1) as fix_pool,
            tc.tile_pool(name="psum", bufs=1, space="PSUM") as psum_pool,
        ):
            ones = fix_pool.tile([P, 1], BF16)
            neg_ones = fix_pool.tile([P, 1], BF16)
            bias_nhalf = fix_pool.tile([P, 1], F32)
            nc.vector.memset(ones[:, :], 1.0)
            nc.vector.memset(neg_ones[:, :], -1.0)
            nc.vector.memset(bias_nhalf[:, :], -0.5)
            qsum = psum_pool.tile([1, MM_N], F32)
            res = fix_pool.tile([1, MM_N], F32)
            acc2 = fix_pool.tile([P, N_COLS], F32)

            mm = 0
            xs = ts = None
            cur_dma = -1
            for col, (di, loff, fd, use_act_min) in enumerate(COMP_SEGS):
                if di != cur_dma:
                    ti, off, dfd = DMA_SEGS[di]
                    xs = iox_pool.tile([P, FD], F32, tag="x")
                    ts = iot_pool.tile([P, FD], F32, tag="t")
                    nc.sync.dma_start(xs[:, :dfd], x_t[ti][:, off : off + dfd])
                    nc.sync.dma_start(ts[:, :dfd], t_t[ti][:, off : off + dfd])
                    cur_dma = di
                xv = xs[:, loff : loff + fd]
                tv = ts[:, loff : loff + fd]

                c = mid_pool.tile([P, HFD], BF16, tag="c")
                Ftile = mid_pool.tile([P, HFD], BF16, tag="F")
                x2h = mid_pool.tile([P, HFD], BF16, tag="x2h")
                e = mid_pool.tile([P, HFD], BF16, tag="e")
                S5 = mid_pool.tile([P, HFD], BF16, tag="S5")
                qp = mid_pool.tile([P, HFD], BF16, tag="e")

                y = mid_pool.tile([P, HFD], F32, tag="y")
                # y = (x - 0.5) + M = floor(x) + M  (first: feeds the long chain)
                nc.vector.tensor_scalar(y[:, :fd], xv, 0.5, MAGIC, sub, add)
                # F = y - M  (Copy allows float bias; exact small int)
                nc.scalar.activation(Ftile[:, :fd], y[:, :fd], Copy, bias=-MAGIC, scale=1.0)
                # c = RNE(t) (exact small int, bf16-exact)
                nc.vector.tensor_scalar(c[:, :fd], tv, MAGIC, MAGIC, add, sub)
                # x2h = 2x - 0.5
                nc.scalar.activation(x2h[:, :fd], xv, Copy, bias=-0.5, scale=2.0)
                # e = c + F (exact)
                nc.vector.tensor_tensor(e[:, :fd], c[:, :fd], Ftile[:, :fd], add)
                # S5 = x2h - e = S - 0.5
                nc.vector.tensor_tensor(S5[:, :fd], x2h[:, :fd], e[:, :fd], sub)
                # qp = max(S5, 0.5); PE accumulates +column sums
                nc.vector.tensor_scalar(qp[:, :fd], S5[:, :fd], 0.5, None, amax)
                for k in range(fd // MM_N):
                    nc.tensor.matmul(
                        qsum[:, :], ones[:, :], qp[:, k * MM_N : (k + 1) * MM_N],
                        start=(mm == 0), stop=(mm == n_matmuls - 1),
                    )
                    mm += 1
                if use_act_min:
                    # acc2 = sum relu(-S5 - 0.5) = sum max(-S5,.5) - fd/2
                    nc.scalar.activation(
                        x2h[:, :fd], S5[:, :fd], Relu,
                        bias=bias_nhalf[:, :], scale=-1.0,
                        accum_out=acc2[:, col : col + 1],
                    )
                else:
                    # qm = min(S5, -0.5); PE accumulates -column sums
                    qm = mid_pool.tile([P, HFD], BF16, tag="e")
                    nc.vector.tensor_scalar(qm[:, :fd], S5[:, :fd], -0.5, None, amin)
                    for k in range(fd // MM_N):
                        nc.tensor.matmul(
                            qsum[:, :], neg_ones[:, :], qm[:, k * MM_N : (k + 1) * MM_N],
                            start=(mm == 0), stop=(mm == n_matmuls - 1),
                        )
                        mm += 1

            nc.vector.tensor_copy(res[:, :], qsum[:, :])
            nc.sync.dma_start(qsum_d[:, :], res[:, :])
            nc.sync.dma_start(acc2_d[:, :], acc2[:, :])

    nc.compile()
    return nc


def kernel(input, target):
    global LAST_RESULTS
    x = np.ascontiguousarray(np.asarray(input, dtype=np.float32))
    t = np.ascontiguousarray(np.asarray(target, dtype=np.float32))
    assert x.shape == (B, D) and t.shape == (B, D)

    if "nc" not in _CACHE:
        _CACHE["nc"] = build_nc()
    nc = _CACHE["nc"]

    in_maps = []
    for j in range(N_CORES):
        r0, r1 = j * ROWS, (j + 1) * ROWS
        in_maps.append(
            {
                "input": np.ascontiguousarray(x[r0:r1]),
                "target": np.ascontiguousarray(t[r0:r1]),
            }
        )

    res = run_bass_kernel_spmd(nc, in_maps, core_ids=list(range(N_CORES)))
    LAST_RESULTS = res

    # acc2 is only written for ACT-min segments — mask the rest (output
    # buffers are NOT zero-initialized).
    act_cols = np.array([ua for (_, _, _, ua) in COMP_SEGS], dtype=bool)
    q_total = 0.0
    s2 = 0.0
    for j in range(N_CORES):
        q_total += res.results[j]["qsum"].astype(np.float64).sum()
        s2 += res.results[j]["acc2"].astype(np.float64)[:, act_cols].sum()

    # sum(loss) = sum max(S5,.5) + sum max(-S5,.5) - N
    #   q_total = sum max(S5,.5) - sum_{DVE segs} min(S5,-.5)
    #   s2      = sum_{ACT segs} max(-S5,.5) - ACT_MIN_ELEMS/2   (per core)
    n = float(B) * float(D)
    loss = (q_total + s2 + N_CORES * ACT_MIN_ELEMS / 2.0 - n) / n
    return np.float32(loss)


# revision 62
# speedup vs baseline: 1.1773x; 1.1773x over previous
"""Distributed Trainium2 kernel for nn_ARLoss_88390426951926.

Computes mean(loss) where, per element (EPS = 1e-6):
    c = round(t); d = x - c; pos = d >= 0
    z = pos ? ceil(x) : floor(x)
    loss = max(0, |d| - |x - z| + pos*EPS)

Algebraic reduction (validated to ~5e-6 rel err on the real data):
    With F = floor(x), S = 2x - c - F, S5 = S - 0.5:
        loss = relu(S - 1 + eps) + relu(-S)            (ties measure-zero)
             = max(S5, 0.5) + max(-S5, 0.5) - 1
so sum(loss) = sum max(S5,.5) - sum min(S5,-.5) - N.

Rounding uses the float32 magic-number trick (M = 1.5*2^23):
    c = (t + M) - M            (round-to-nearest-even)
    F = ((x - 0.5) + M) - M    (floor, up to RNE ties at exact integers)

Engine split per 128xFD tile (order matters: the long chain is
x -> y -> F -> e -> S5, so x's DMA is issued first and y/F run first):
    DVE TS2: y   = (x - 0.5) + M       f32->f32   (2x_2p, 0.5 cyc/elem)
    ACT A1 : F   = Copy(y - M)         f32->bf16
    DVE TS1: c   = (t + M) - M         f32->bf16  (2x_2p)
    ACT A2 : x2h = Copy(2x - 0.5)      f32->bf16
    DVE TT1: e   = c + F               bf16       (2x_1p)
    DVE TT2: S5  = x2h - e             bf16       (2x_1p)
    DVE TSp: qp  = max(S5, 0.5)        bf16       (4x_2p)
    PE     : psum[1,512] += ones.T @ qp            (idle engine)
    min-branch, alternating per segment to balance DVE vs ACT:
      DVE TSm: qm = min(S5, -0.5); psum += (-ones).T @ qm
      ACT AR : acc2[:,col] = sum relu(-S5 - 0.5)   (accum_out, full rate)
First and last tiles are split into 4 quarter tiles so the pipeline
ramps in and drains out quickly. DMA granularity stays at 2 MiB per
transfer, but compute runs in [128, 2048] half-tiles, halving the mid
pool so the x-input pool affords 5 buffers (~10 MiB of DMA lookahead);
this keeps the input stream gapless (measured: continuous 8.8->93.5us).
Per core measured: DVE ~93us busy, ACT ~83us, PE ~43us, DMA ~94us (HBM
roofline); exec ~118us = DMA + ~21us fixed Tile prologue/epilogue + ~3us
residual. Device intermittently clock-throttles ~20% (then ~140us).
Notes from hardware probing (this toolchain):
  - tensor_scalar with accum_out (cache-reduce form) runs at ~1.2
    cyc/elem regardless of dtype - do NOT use it on the hot path.
  - abs_max is not a valid ALU op on HW TensorScalar; max/min are.
  - ACT activation accum_out runs at full ACT pass rate.
"""

import sys
import types

import numpy as np

import concourse.bass as bass
import concourse.bacc as bacc
import concourse.mybir as mybir
from concourse.tile import TileContext
from concourse.bass_utils import run_bass_kernel_spmd


def _ensure_axon_hooks():
    """Some agent images lack ``antenv.axon_hooks``; if BASS_TRACE is set
    in the environment, run_bass_kernel_spmd imports it and would crash.
    Provide a no-op hook registry so tracing degrades gracefully."""
    try:
        import antenv  # noqa: F401
    except ImportError:
        return
    try:
        import antenv.axon_hooks  # noqa: F401
        return
    except ImportError:
        pass
    mod = types.ModuleType("antenv.axon_hooks")
    _state = {"hook": None}
    mod.set_axon_ntff_profile_hook = lambda h: _state.__setitem__("hook", h)
    mod.get_axon_ntff_profile_hook = lambda: _state["hook"]
    sys.modules["antenv.axon_hooks"] = mod
    import antenv as _a

    _a.axon_hooks = mod


_ensure_axon_hooks()

B, D = 8192, 4096
N_CORES = 8
ROWS = B // N_CORES          # 1024 rows per core
P = 128                      # SBUF partitions
FD = 4096                    # free dim per full tile
NTILES = (ROWS * D) // (P * FD)   # 8 full tiles per core
MAGIC = 12582912.0           # 1.5 * 2**23
MM_N = 512                   # matmul free-dim chunk (one PSUM bank)

F32 = mybir.dt.float32
BF16 = mybir.dt.bfloat16

HFD = FD // 2                # compute half-tile width (smaller mid pool)

# DMA segments: (tile_idx, fd_off, dma_fd). First/last tiles in quarters
# for fast ramp/drain; middle tiles as full 2 MiB transfers.
DMA_SEGS = [(0, s * (FD // 4), FD // 4) for s in range(4)]
DMA_SEGS += [(i, 0, FD) for i in range(1, NTILES - 1)]
DMA_SEGS += [(NTILES - 1, s * (FD // 4), FD // 4) for s in range(4)]

# Compute segments: (dma_idx, local_off, fd, use_act_min). Full tiles are
# computed in two half-tiles so mid intermediates are [P, HFD].
COMP_SEGS = []
for _di, (_ti, _off, _dfd) in enumerate(DMA_SEGS):
    _ua = _ti in (2, 4, 6) if _dfd == FD else (_ti == NTILES - 1)
    for _h in range(max(1, _dfd // HFD)):
        COMP_SEGS.append((_di, _h * HFD, min(HFD, _dfd), _ua))
N_COLS = len(COMP_SEGS)
# ACT relu-accum cells compute max(-S5,.5) - 0.5 per element.
ACT_MIN_ELEMS = sum(P * fd for (_, _, fd, ua) in COMP_SEGS if ua)

# Exposed for test.py: the BassKernelResults of the last run.
LAST_RESULTS = None
_CACHE = {}


def build_nc():
    nc = bacc.Bacc(dynamic_dma_scratch_size=512)
    x_d = nc.dram_tensor("input", [ROWS, D], F32, kind="ExternalInput")
    t_d = nc.dram_tensor("target", [ROWS, D], F32, kind="ExternalInput")
    qsum_d = nc.dram_tensor("qsum", [1, MM_N], F32, kind="ExternalOutput")
    acc2_d = nc.dram_tensor("acc2", [P, N_COLS], F32, kind="ExternalOutput")

    x_t = x_d[:, :].rearrange("(n p) m -> n p m", p=P)
    t_t = t_d[:, :].rearrange("(n p) m -> n p m", p=P)

    add = mybir.AluOpType.add
    sub = mybir.AluOpType.subtract
    amax = mybir.AluOpType.max
    amin = mybir.AluOpType.min
    Copy = mybir.ActivationFunctionType.Copy
    Relu = mybir.ActivationFunctionType.Relu

    n_matmuls = sum(
        (fd // MM_N) * (1 if ua else 2) for (_, _, fd, ua) in COMP_SEGS
    )

    with TileContext(nc) as tc:
        with (
            tc.tile_pool(name="iox", bufs=5) as iox_pool,
            tc.tile_pool(name="iot", bufs=3) as iot_pool,
            tc.tile_pool(name="mid", bufs=2) as mid_pool,
            tc.tile_pool(name="prod", bufs=3) as prod_pool,
            tc.tile_pool(name="fix", bufs=1) as fix_pool,
            tc.tile_pool(name="psum", bufs=1, space="PSUM") as psum_pool,
        ):
            ones = fix_pool.tile([P, 1], BF16)
            neg_ones = fix_pool.tile([P, 1], BF16)
            bias_nhalf = fix_pool.tile([P, 1], F32)
            nc.vector.memset(ones[:, :], 1.0)
            nc.vector.memset(neg_ones[:, :], -1.0)
            nc.vector.memset(bias_nhalf[:, :], -0.5)
            qsum = psum_pool.tile([1, MM_N], F32)
            res = fix_pool.tile([1, MM_N], F32)
            acc2 = fix_pool.tile([P, N_COLS], F32)

            mm = 0
            xs = ts = None
            cur_dma = -1
            for col, (di, loff, fd, use_act_min) in enumerate(COMP_SEGS):
                if di != cur_dma:
                    ti, off, dfd = DMA_SEGS[di]
                    xs = iox_pool.tile([P, FD], F32, tag="x")
                    ts = iot_pool.tile([P, FD], F32, tag="t")
                    nc.sync.dma_start(xs[:, :dfd], x_t[ti][:, off : off + dfd])
                    nc.sync.dma_start(ts[:, :dfd], t_t[ti][:, off : off + dfd])
                    cur_dma = di
                xv = xs[:, loff : loff + fd]
                tv = ts[:, loff : loff + fd]

                c = prod_pool.tile([P, HFD], BF16, tag="c")
                Ftile = mid_pool.tile([P, HFD], BF16, tag="F")
                x2h = mid_pool.tile([P, HFD], BF16, tag="x2h")
                e = mid_pool.tile([P, HFD], BF16, tag="e")
                S5 = mid_pool.tile([P, HFD], BF16, tag="S5")
                qp = mid_pool.tile([P, HFD], BF16, tag="e")

                y = prod_pool.tile([P, HFD], F32, tag="y")
                # y = (x - 0.5) + M = floor(x) + M  (first: feeds the long chain)
                nc.vector.tensor_scalar(y[:, :fd], xv, 0.5, MAGIC, sub, add)
                # F = y - M  (Copy allows float bias; exact small int)
                nc.scalar.activation(Ftile[:, :fd], y[:, :fd], Copy, bias=-MAGIC, scale=1.0)
                # c = RNE(t) (exact small int, bf16-exact)
                nc.vector.tensor_scalar(c[:, :fd], tv, MAGIC, MAGIC, add, sub)
                # x2h = 2x - 0.5
                nc.scalar.activation(x2h[:, :fd], xv, Copy, bias=-0.5, scale=2.0)
                # e = c + F (exact)
                nc.vector.tensor_tensor(e[:, :fd], c[:, :fd], Ftile[:, :fd], add)
                # S5 = x2h - e = S - 0.5
                nc.vector.tensor_tensor(S5[:, :fd], x2h[:, :fd], e[:, :fd], sub)
                # qp = max(S5, 0.5); PE accumulates +column sums
                nc.vector.tensor_scalar(qp[:, :fd], S5[:, :fd], 0.5, None, amax)
                for k in range(fd // MM_N):
                    nc.tensor.matmul(
                        qsum[:, :], ones[:, :], qp[:, k * MM_N : (k + 1) * MM_N],
                        start=(mm == 0), stop=(mm == n_matmuls - 1),
                    )
                    mm += 1
                if use_act_min:
                    # acc2 = sum relu(-S5 - 0.5) = sum max(-S5,.5) - fd/2
                    nc.scalar.activation(
                        x2h[:, :fd], S5[:, :fd], Relu,
                        bias=bias_nhalf[:, :], scale=-1.0,
                        accum_out=acc2[:, col : col + 1],
                    )
                else:
                    # qm = min(S5, -0.5); PE accumulates -column sums
                    qm = mid_pool.tile([P, HFD], BF16, tag="e")
                    nc.vector.tensor_scalar(qm[:, :fd], S5[:, :fd], -0.5, None, amin)
                    for k in range(fd // MM_N):
                        nc.tensor.matmul(
                            qsum[:, :], neg_ones[:, :], qm[:, k * MM_N : (k + 1) * MM_N],
                            start=(mm == 0), stop=(mm == n_matmuls - 1),
                        )
                        mm += 1

            nc.scalar.copy(res[:, :], qsum[:, :])
            nc.sync.dma_start(qsum_d[:, :], res[:, :])
            nc.sync.dma_start(acc2_d[:, :], acc2[:, :])

    nc.compile()
    return nc


def kernel(input, target):
    global LAST_RESULTS
    x = np.ascontiguousarray(np.asarray(input, dtype=np.float32))
    t = np.ascontiguousarray(np.asarray(target, dtype=np.float32))
    assert x.shape == (B, D) and t.shape == (B, D)

    if "nc" not in _CACHE:
        _CACHE["nc"] = build_nc()
    nc = _CACHE["nc"]

    in_maps = []
    for j in range(N_CORES):
        r0, r1 = j * ROWS, (j + 1) * ROWS
        in_maps.append(
            {
                "input": np.ascontiguousarray(x[r0:r1]),
                "target": np.ascontiguousarray(t[r0:r1]),
            }
        )

    res = run_bass_kernel_spmd(nc, in_maps, core_ids=list(range(N_CORES)))
    LAST_RESULTS = res

    # acc2 is only written for ACT-min segments — mask the rest (output
    # buffers are NOT zero-initialized).
    act_cols = np.array([ua for (_, _, _, ua) in COMP_SEGS], dtype=bool)
    q_total = 0.0
    s2 = 0.0
    for j in range(N_CORES):
        q_total += res.results[j]["qsum"].astype(np.float64).sum()
        s2 += res.results[j]["acc2"].astype(np.float64)[:, act_cols].sum()

    # sum(loss) = sum max(S5,.5) + sum max(-S5,.5) - N
    #   q_total = sum max(S5,.5) - sum_{DVE segs} min(S5,-.5)
    #   s2      = sum_{ACT segs} max(-S5,.5) - ACT_MIN_ELEMS/2   (per core)
    n = float(B) * float(D)
    loss = (q_total + s2 + N_CORES * ACT_MIN_ELEMS / 2.0 - n) / n
    return np.float32(loss)


# revision 63
# speedup vs baseline: 1.1792x; 1.0016x over previous
"""Distributed Trainium2 kernel for nn_ARLoss_88390426951926.

Computes mean(loss) where, per element (EPS = 1e-6):
    c = round(t); d = x - c; pos = d >= 0
    z = pos ? ceil(x) : floor(x)
    loss = max(0, |d| - |x - z| + pos*EPS)

Algebraic reduction (validated to ~5e-6 rel err on the real data):
    With F = floor(x), S = 2x - c - F, S5 = S - 0.5:
        loss = relu(S - 1 + eps) + relu(-S)            (ties measure-zero)
             = max(S5, 0.5) + max(-S5, 0.5) - 1
so sum(loss) = sum max(S5,.5) - sum min(S5,-.5) - N.

Rounding uses the float32 magic-number trick (M = 1.5*2^23):
    c = (t + M) - M            (round-to-nearest-even)
    F = ((x - 0.5) + M) - M    (floor, up to RNE ties at exact integers)

Engine split per 128xFD tile (order matters: the long chain is
x -> y -> F -> e -> S5, so x's DMA is issued first and y/F run first):
    DVE TS2: y   = (x - 0.5) + M       f32->f32   (2x_2p, 0.5 cyc/elem)
    ACT A1 : F   = Copy(y - M)         f32->bf16
    DVE TS1: c   = (t + M) - M         f32->bf16  (2x_2p)
    ACT A2 : x2h = Copy(2x - 0.5)      f32->bf16
    DVE TT1: e   = c + F               bf16       (2x_1p)
    DVE TT2: S5  = x2h - e             bf16       (2x_1p)
    DVE TSp: qp  = max(S5, 0.5)        bf16       (4x_2p)
    PE     : psum[1,512] += ones.T @ qp            (idle engine)
    min-branch, alternating per segment to balance DVE vs ACT:
      DVE TSm: qm = min(S5, -0.5); psum += (-ones).T @ qm
      ACT AR : acc2[:,col] = sum relu(-S5 - 0.5)   (accum_out, full rate)
First and last tiles are split into 4 quarter tiles so the pipeline
ramps in and drains out quickly.
Per core measured: DVE ~85us, ACT ~75us, PE ~11us, DMA ~94us (HBM
roofline); exec ~122us = DMA + ~21us fixed Tile prologue/epilogue.
Notes from hardware probing (this toolchain):
  - tensor_scalar with accum_out (cache-reduce form) runs at ~1.2
    cyc/elem regardless of dtype - do NOT use it on the hot path.
  - abs_max is not a valid ALU op on HW TensorScalar; max/min are.
  - ACT activation accum_out runs at full ACT pass rate.
"""

import sys
import types

import numpy as np

import concourse.bass as bass
import concourse.bacc as bacc
import concourse.mybir as mybir
from concourse.tile import TileContext
from concourse.bass_utils import run_bass_kernel_spmd


def _ensure_axon_hooks():
    """Some agent images lack ``antenv.axon_hooks``; if BASS_TRACE is set
    in the environment, run_bass_kernel_spmd imports it and would crash.
    Provide a no-op hook registry so tracing degrades gracefully."""
    try:
        import antenv  # noqa: F401
    except ImportError:
        return
    try:
        import antenv.axon_hooks  # noqa: F401
        return
    except ImportError:
        pass
    mod = types.ModuleType("antenv.axon_hooks")
    _state = {"hook": None}
    mod.set_axon_ntff_profile_hook = lambda h: _state.__setitem__("hook", h)
    mod.get_axon_ntff_profile_hook = lambda: _state["hook"]
    sys.modules["antenv.axon_hooks"] = mod
    import antenv as _a

    _a.axon_hooks = mod


_ensure_axon_hooks()

B, D = 8192, 4096
N_CORES = 8
ROWS = B // N_CORES          # 1024 rows per core
P = 128                      # SBUF partitions
FD = 4096                    # free dim per full tile
NTILES = (ROWS * D) // (P * FD)   # 8 full tiles per core
MAGIC = 12582912.0           # 1.5 * 2**23
MM_N = 512                   # matmul free-dim chunk (one PSUM bank)

F32 = mybir.dt.float32
BF16 = mybir.dt.bfloat16

HFD = FD // 2                # compute half-tile width (smaller mid pool)

# DMA segments: (tile_idx, fd_off, dma_fd). First/last tiles in quarters
# for fast ramp/drain; middle tiles as full 2 MiB transfers.
DMA_SEGS = [(0, s * (FD // 4), FD // 4) for s in range(4)]
DMA_SEGS += [(i, 0, FD) for i in range(1, NTILES - 1)]
DMA_SEGS += [(NTILES - 1, s * (FD // 4), FD // 4) for s in range(4)]

# Compute segments: (dma_idx, local_off, fd, use_act_min). Full tiles are
# computed in two half-tiles so mid intermediates are [P, HFD].
COMP_SEGS = []
for _di, (_ti, _off, _dfd) in enumerate(DMA_SEGS):
    _ua = _ti in (2, 4, 6) if _dfd == FD else (_ti == NTILES - 1)
    for _h in range(max(1, _dfd // HFD)):
        COMP_SEGS.append((_di, _h * HFD, min(HFD, _dfd), _ua))
N_COLS = len(COMP_SEGS)
# ACT relu-accum cells compute max(-S5,.5) - 0.5 per element.
ACT_MIN_ELEMS = sum(P * fd for (_, _, fd, ua) in COMP_SEGS if ua)

# Exposed for test.py: the BassKernelResults of the last run.
LAST_RESULTS = None
_CACHE = {}


def build_nc():
    nc = bacc.Bacc(dynamic_dma_scratch_size=512)
    x_d = nc.dram_tensor("input", [ROWS, D], F32, kind="ExternalInput")
    t_d = nc.dram_tensor("target", [ROWS, D], F32, kind="ExternalInput")
    qsum_d = nc.dram_tensor("qsum", [1, MM_N], F32, kind="ExternalOutput")
    acc2_d = nc.dram_tensor("acc2", [P, N_COLS], F32, kind="ExternalOutput")

    x_t = x_d[:, :].rearrange("(n p) m -> n p m", p=P)
    t_t = t_d[:, :].rearrange("(n p) m -> n p m", p=P)

    add = mybir.AluOpType.add
    sub = mybir.AluOpType.subtract
    amax = mybir.AluOpType.max
    amin = mybir.AluOpType.min
    Copy = mybir.ActivationFunctionType.Copy
    Relu = mybir.ActivationFunctionType.Relu

    n_matmuls = sum(
        (fd // MM_N) * (1 if ua else 2) for (_, _, fd, ua) in COMP_SEGS
    )

    with TileContext(nc) as tc:
        with (
            tc.tile_pool(name="iox", bufs=5) as iox_pool,
            tc.tile_pool(name="iot", bufs=3) as iot_pool,
            tc.tile_pool(name="mid", bufs=2) as mid_pool,
            tc.tile_pool(name="fix", bufs=1) as fix_pool,
            tc.tile_pool(name="psum", bufs=1, space="PSUM") as psum_pool,
        ):
            ones = fix_pool.tile([P, 1], BF16)
            neg_ones = fix_pool.tile([P, 1], BF16)
            bias_nhalf = fix_pool.tile([P, 1], F32)
            nc.vector.memset(ones[:, :], 1.0)
            nc.vector.memset(neg_ones[:, :], -1.0)
            nc.vector.memset(bias_nhalf[:, :], -0.5)
            qsum = psum_pool.tile([1, MM_N], F32)
            res = fix_pool.tile([1, MM_N], F32)
            acc2 = fix_pool.tile([P, N_COLS], F32)

            mm = 0
            xs = ts = None
            cur_dma = -1
            for col, (di, loff, fd, use_act_min) in enumerate(COMP_SEGS):
                if di != cur_dma:
                    ti, off, dfd = DMA_SEGS[di]
                    xs = iox_pool.tile([P, FD], F32, tag="x")
                    ts = iot_pool.tile([P, FD], F32, tag="t")
                    nc.sync.dma_start(xs[:, :dfd], x_t[ti][:, off : off + dfd])
                    nc.sync.dma_start(ts[:, :dfd], t_t[ti][:, off : off + dfd])
                    cur_dma = di
                xv = xs[:, loff : loff + fd]
                tv = ts[:, loff : loff + fd]

                c = mid_pool.tile([P, HFD], BF16, tag="c")
                Ftile = mid_pool.tile([P, HFD], BF16, tag="F")
                x2h = mid_pool.tile([P, HFD], BF16, tag="x2h")
                e = mid_pool.tile([P, HFD], BF16, tag="e")
                S5 = mid_pool.tile([P, HFD], BF16, tag="S5")
                qp = mid_pool.tile([P, HFD], BF16, tag="e")

                y = mid_pool.tile([P, HFD], F32, tag="y")
                # y = (x - 0.5) + M = floor(x) + M  (first: feeds the long chain)
                nc.vector.tensor_scalar(y[:, :fd], xv, 0.5, MAGIC, sub, add)
                # F = y - M  (Copy allows float bias; exact small int)
                nc.scalar.activation(Ftile[:, :fd], y[:, :fd], Copy, bias=-MAGIC, scale=1.0)
                # c = RNE(t) (exact small int, bf16-exact)
                nc.vector.tensor_scalar(c[:, :fd], tv, MAGIC, MAGIC, add, sub)
                # x2h = 2x - 0.5
                nc.scalar.activation(x2h[:, :fd], xv, Copy, bias=-0.5, scale=2.0)
                # e = c + F (exact)
                nc.vector.tensor_tensor(e[:, :fd], c[:, :fd], Ftile[:, :fd], add)
                # S5 = x2h - e = S - 0.5
                nc.vector.tensor_tensor(S5[:, :fd], x2h[:, :fd], e[:, :fd], sub)
                # qp = max(S5, 0.5); PE accumulates +column sums
                nc.vector.tensor_scalar(qp[:, :fd], S5[:, :fd], 0.5, None, amax)
                for k in range(fd // MM_N):
                    nc.tensor.matmul(
                        qsum[:, :], ones[:, :], qp[:, k * MM_N : (k + 1) * MM_N],
                        start=(mm == 0), stop=(mm == n_matmuls - 1),
                    )
                    mm += 1
                if use_act_min:
                    # acc2 = sum relu(-S5 - 0.5) = sum max(-S5,.5) - fd/2
                    nc.scalar.activation(
                        x2h[:, :fd], S5[:, :fd], Relu,
                        bias=bias_nhalf[:, :], scale=-1.0,
                        accum_out=acc2[:, col : col + 1],
                    )
                else:
                    # qm = min(S5, -0.5); PE accumulates -column sums
                    qm = mid_pool.tile([P, HFD], BF16, tag="e")
                    nc.vector.tensor_scalar(qm[:, :fd], S5[:, :fd], -0.5, None, amin)
                    for k in range(fd // MM_N):
                        nc.tensor.matmul(
                            qsum[:, :], neg_ones[:, :], qm[:, k * MM_N : (k + 1) * MM_N],
                            start=(mm == 0), stop=(mm == n_matmuls - 1),
                        )
                        mm += 1

            nc.scalar.copy(res[:, :], qsum[:, :])
            nc.sync.dma_start(qsum_d[:, :], res[:, :])
            nc.sync.dma_start(acc2_d[:, :], acc2[:, :])

    nc.compile()
    return nc


def kernel(input, target):
    global LAST_RESULTS
    x = np.ascontiguousarray(np.asarray(input, dtype=np.float32))
    t = np.ascontiguousarray(np.asarray(target, dtype=np.float32))
    assert x.shape == (B, D) and t.shape == (B, D)

    if "nc" not in _CACHE:
        _CACHE["nc"] = build_nc()
    nc = _CACHE["nc"]

    in_maps = []
    for j in range(N_CORES):
        r0, r1 = j * ROWS, (j + 1) * ROWS
        in_maps.append(
            {
                "input": np.ascontiguousarray(x[r0:r1]),
                "target": np.ascontiguousarray(t[r0:r1]),
            }
        )

    res = run_bass_kernel_spmd(nc, in_maps, core_ids=list(range(N_CORES)))
    LAST_RESULTS = res

    # acc2 is only written for ACT-min segments — mask the rest (output
    # buffers are NOT zero-initialized).
    act_cols = np.array([ua for (_, _, _, ua) in COMP_SEGS], dtype=bool)
    q_total = 0.0
    s2 = 0.0
    for j in range(N_CORES):
        q_total += res.results[j]["qsum"].astype(np.float64).sum()
        s2 += res.results[j]["acc2"].astype(np.float64)[:, act_cols].sum()

    # sum(loss) = sum max(S5,.5) + sum max(-S5,.5) - N
    #   q_total = sum max(S5,.5) - sum_{DVE segs} min(S5,-.5)
    #   s2      = sum_{ACT segs} max(-S5,.5) - ACT_MIN_ELEMS/2   (per core)
    n = float(B) * float(D)
    loss = (q_total + s2 + N_CORES * ACT_MIN_ELEMS / 2.0 - n) / n
    return np.float32(loss)
